# revision 16
# baseline (speedup 1.0000x reference)
"""EnhancedDynamicChannelAttention Trainium2 kernel (bf16 datapath, v4).

Reference computation (B=16, S=2048, C=1024, H=8, HD=128):
    q[b,h,:]   = pref[b,h]*Wq[:,0] + bq
    k          = f @ Wk.T + bk ;  v = f @ Wv.T + bv       (per head slice)
    scores     = softmax_s(q . k)                          [B,H,S]
    ctx[b,h,:] = sum_s scores * v[b,s,h,:]                 [B,H,HD]
    out        = f + broadcast_s(ctx)

Algebraic folding (exact up to fp reassociation):
  - softmax shift invariance  -> the q.bk term drops entirely.
  - scores[b,h,s] = f[b,s,h,:] . qk[b,h,:]  with  qk = (pref*Wq+bq) @ Wk
  - sum_s attn = 1  ->  ctx = Wv @ (sum_s attn*f[b,s,h,:]) + bv
  So k/v are never materialized; the kernel is memory bound.

Trace-driven design (v2/v3 perfetto analysis):
  - bf16 datapath end to end (harness gate 2e-2; measured ~5e-3).
  - One HWDGE ring sustains only ~170-210 GB/s, so every f load/store
    is split into half-supertile (0.5 MiB) DMAs alternating between
    the SP and ACT rings; small constants ride the GPSIMD SWDGE ring.
  - DVE and GPSIMD arbitrate one shared SBUF port pair (exclusive
    lock), so ALL elementwise work stays on DVE: tensor_tensor is
    2 elem/cycle in bf16, tensor_reduce always 1 elem/cycle; the
    128-wide segmented reduce is two bf16 TT halvings + 32-wide reduce.
  - 1/sumE folds into the PSUM->SBUF copies (ACT scale operand); bv
    and the ctx partition broadcast are K=1 PE matmuls (no DRAM
    bounce).  Batch-0's tail is ACT-only (DVE is busy streaming
    batch 1); batch-1's tail splits across ACT and DVE chains.
  - Residual adds are DVE out-of-place into a 4-deep staging pool so
    adds never stall on store completion.

Distribution: pure data parallel over batch, 2 batches per core, 8 cores.
"""

import numpy as np

B, S, C = 16, 2048, 1024
H, HD = 8, 128
N_CORES = 8
BPC = B // N_CORES          # batches per core
ST = 4                      # s-rows per partition in a super tile
P = 128
SUP = S // (P * ST)         # super tiles per batch (4)
NT = S // P                 # sub tiles per batch (16)

_CACHE = {}


def _build_program():
    import concourse.bass as bass
    import concourse.bacc as bacc
    import concourse.tile as tile
    from concourse import mybir

    f32 = mybir.dt.float32
    bf16 = mybir.dt.bfloat16

    nc = bacc.Bacc("TRN2", debug=False, num_devices=N_CORES)
    f_in = nc.dram_tensor("features", [BPC, S, C], bf16, kind="ExternalInput")
    qk_in = nc.dram_tensor("qkflat", [BPC, C], bf16, kind="ExternalInput")
    wvt_in = nc.dram_tensor("wvt", [HD, HD], bf16, kind="ExternalInput")
    bvf_in = nc.dram_tensor("bvflat", [1, C], bf16, kind="ExternalInput")
    id8_in = nc.dram_tensor("ident8", [8, 8], bf16, kind="ExternalInput")
    ones_in = nc.dram_tensor("ones128", [P, 1], bf16, kind="ExternalInput")
    onesr_in = nc.dram_tensor("onesrow", [1, P], bf16, kind="ExternalInput")
    out_t = nc.dram_tensor("out", [BPC, S, C], bf16, kind="ExternalOutput")

    with tile.TileContext(nc) as tc, nc.allow_low_precision(
        reason="bf16 datapath; rel-err budget 2e-2"
    ):
        with (
            tc.tile_pool(name="fpool", bufs=BPC) as fpool,
            tc.tile_pool(name="tmppool", bufs=2) as tmppool,
            tc.tile_pool(name="opool", bufs=4) as opool,
            tc.tile_pool(name="spool", bufs=2 * SUP) as spool,
            tc.tile_pool(name="small", bufs=2) as small,
            tc.tile_pool(name="singles", bufs=1) as singles,
            tc.tile_pool(name="ps1", bufs=1, space="PSUM") as ps1,
            tc.tile_pool(name="ps2", bufs=2, space="PSUM") as ps2,
        ):
            # tiny loads first: onesrow + qk rows (single-descriptor DMAs);
            # the DMA-broadcast of qk (128 dup reads) is slow on any ring,
            # so broadcast on-chip with K=1 PE matmuls instead.  Other
            # constants ride the GPSIMD SWDGE ring.
            onesr_sb = singles.tile([1, P], bf16)
            nc.sync.dma_start(out=onesr_sb, in_=onesr_in[:, :])
            qk_rows = []
            for b in range(BPC):
                qk_row = small.tile([1, C], bf16, tag="qkrow", name=f"qkr{b}")
                nc.scalar.dma_start(out=qk_row, in_=qk_in[b : b + 1, :])
                qk_rows.append(qk_row)

            ones_sb = singles.tile([P, 1], bf16)
            nc.gpsimd.dma_start(out=ones_sb, in_=ones_in[:, :])
            wvt_sb = singles.tile([HD, HD], bf16)
            nc.gpsimd.dma_start(out=wvt_sb, in_=wvt_in[:, :])
            id8_sb = singles.tile([8, 8], bf16)
            nc.gpsimd.dma_start(out=id8_sb, in_=id8_in[:, :])
            bvf_sb = singles.tile([1, C], bf16)
            nc.gpsimd.dma_start(out=bvf_sb, in_=bvf_in[:, :])

            # qk_bc[p, c] = qk[c] for all p, via ones (x) row PE matmuls
            qk_bcs = []
            for b in range(BPC):
                qk_bc = small.tile([P, C], bf16, tag="qkbc", name=f"qkbc{b}")
                for half in range(2):
                    cs = slice(half * 512, (half + 1) * 512)
                    qps = ps1.tile(
                        [P, 512], f32, tag="tailps", bufs=2, name=f"qps{b}{half}"
                    )
                    nc.tensor.matmul(
                        qps, onesr_sb, qk_rows[b][0:1, cs], start=True, stop=True
                    )
                    nc.scalar.copy(out=qk_bc[:, cs], in_=qps)
                qk_bcs.append(qk_bc)

            fbs = [
                fpool.tile([P, NT, C], bf16, tag="fb", name=f"fb{b}")
                for b in range(BPC)
            ]
            uwfAs, uwfBs, sumEs = [], [], []
            for b in range(BPC):
                uwfAs.append(
                    ps2.tile([P, 512], f32, tag="uwfA", bufs=2, name=f"uwfA{b}")
                )
                uwfBs.append(
                    ps2.tile([P, 512], f32, tag="uwfB", bufs=2, name=f"uwfB{b}")
                )
                sumEs.append(
                    ps2.tile([8, 1], f32, tag="sumE", bufs=1, name=f"sumE{b}")
                )
            ctx_bcs = [None] * BPC

            def emit_loads(b):
                # half-supertile (0.5 MiB) DMAs alternating SP/ACT rings:
                # one HWDGE queue alone tops out ~200 GB/s
                fview = f_in[b].rearrange("(st p t) c -> st p t c", p=P, t=ST)
                for st in range(SUP):
                    for half in range(2):
                        lo = st * ST + half * (ST // 2)
                        eng = nc.sync if half == 0 else nc.scalar
                        eng.dma_start(
                            out=fbs[b][:, lo : lo + ST // 2, :],
                            in_=fview[st][
                                :, half * (ST // 2) : (half + 1) * (ST // 2), :
                            ],
                        )

            def emit_stream_unit(b, t_lo, t_hi, first, last):
                # one pipeline unit covering fb rows [t_lo, t_hi)
                nt = t_hi - t_lo
                qk_bc3 = qk_bcs[b].rearrange(
                    "p (o c) -> p o c", o=1
                ).broadcast_to([P, nt, C])
                fb = fbs[b]
                uwfA, uwfB, sumE = uwfAs[b], uwfBs[b], sumEs[b]
                tmp = tmppool.tile([P, nt, C], bf16, tag="tmp")
                nc.vector.tensor_mul(tmp, fb[:, t_lo:t_hi, :], qk_bc3)
                tmp4 = tmp.rearrange("p t (h d) -> p t h d", h=H)
                # segmented reduce over d=128: two packed bf16 halvings
                # (2 elem/cycle) + one 32-wide 1x reduce
                h1 = tmppool.tile([P, nt, H, HD // 2], bf16, tag="h1")
                nc.vector.tensor_add(
                    h1, tmp4[:, :, :, 0 : HD // 2], tmp4[:, :, :, HD // 2 : HD]
                )
                h2 = tmppool.tile([P, nt, H, HD // 4], bf16, tag="h2")
                nc.vector.tensor_add(
                    h2, h1[:, :, :, 0 : HD // 4], h1[:, :, :, HD // 4 : HD // 2]
                )
                h3 = tmppool.tile([P, nt, H, HD // 8], bf16, tag="h3")
                nc.vector.tensor_add(
                    h3, h2[:, :, :, 0 : HD // 8], h2[:, :, :, HD // 8 : HD // 4]
                )
                scores = spool.tile([P, nt, H], f32, tag="scores")
                nc.vector.reduce_sum(scores, h3, axis=mybir.AxisListType.X)
                E_sup = spool.tile([P, nt, H], bf16, tag="esup")
                nc.scalar.activation(
                    out=E_sup.rearrange("p t h -> p (t h)"),
                    in_=scores.rearrange("p t h -> p (t h)"),
                    func=mybir.ActivationFunctionType.Exp,
                )
                for t in range(nt):
                    mm_first = first and t == 0
                    mm_last = last and t == nt - 1
                    e_sl = E_sup[:, t, :]
                    f_sl = fb[:, t_lo + t, :]
                    nc.tensor.matmul(
                        uwfA[0:8, :], e_sl, f_sl[:, 0:512],
                        start=mm_first, stop=mm_last,
                    )
                    nc.tensor.matmul(
                        uwfB[0:8, :], e_sl, f_sl[:, 512:1024],
                        start=mm_first, stop=mm_last,
                    )
                    nc.tensor.matmul(
                        sumE, e_sl, ones_sb, start=mm_first, stop=mm_last
                    )

            def emit_stream(b, st_range, split_last=False):
                for st in st_range:
                    last_st = st == SUP - 1
                    if split_last and last_st:
                        # halve the final unit so the tail's PE drain starts
                        # ~2us earlier
                        emit_stream_unit(
                            b, st * ST, st * ST + ST // 2, st == 0, False
                        )
                        emit_stream_unit(
                            b, st * ST + ST // 2, (st + 1) * ST, False, True
                        )
                    else:
                        emit_stream_unit(
                            b, st * ST, (st + 1) * ST, st == 0, last_st
                        )

            def emit_tail(b, use_dve):
                # wf = diag-blocks(uwf)/sumE; ctx_row = wf @ WvT + bv;
                # partition-broadcast via K=1 PE matmuls (ones (x) row).
                # use_dve=True splits the two C-halves across ACT and DVE
                # chains (only when DVE is otherwise idle, i.e. last batch).
                uwfA, uwfB, sumE = uwfAs[b], uwfBs[b], sumEs[b]
                recip = small.tile([8, 1], f32, tag="recip", name=f"recip{b}")
                nc.vector.reciprocal(recip, sumE)
                uwf_sb = small.tile([8, C], bf16, tag="uwfsb", bufs=1)
                nc.scalar.mul(uwf_sb[:, 0:512], uwfA[0:8, :], recip)
                if use_dve:
                    nc.vector.tensor_scalar_mul(
                        uwf_sb[:, 512:1024], uwfB[0:8, :], recip
                    )
                else:
                    nc.scalar.mul(uwf_sb[:, 512:1024], uwfB[0:8, :], recip)
                # per-head PE transpose into [128, 8*8]; diagonal columns
                # (stride 9) hold wfT[d, h] = uwf[h, h*128+d] / sumE[h]
                wfT8_ps = ps1.tile([P, H * H], bf16, tag="wft8")
                for h in range(H):
                    nc.tensor.transpose(
                        wfT8_ps[:, h * H : (h + 1) * H],
                        uwf_sb[:, h * HD : (h + 1) * HD],
                        id8_sb,
                    )
                wfT8_sb = small.tile([P, H * H], bf16, tag="wft8sb", bufs=1)
                nc.scalar.copy(out=wfT8_sb, in_=wfT8_ps)

                ctx_bc = small.tile([P, C], bf16, tag="ctxbc", name=f"ctxbc{b}")
                for half in range(2):
                    cs = slice(half * 512, (half + 1) * 512)
                    on_dve = use_dve and half == 1
                    tailps = ps1.tile(
                        [P, 512], f32, tag="tailps", bufs=2, name=f"tps{b}{half}"
                    )
                    for hh in range(4):
                        h = half * 4 + hh
                        nc.tensor.matmul(
                            tailps[0:1, hh * HD : (hh + 1) * HD],
                            wfT8_sb[:, h * (H + 1) : h * (H + 1) + 1],
                            wvt_sb,
                            start=True,
                            stop=True,
                        )
                    ctx_row = small.tile(
                        [1, 512], bf16, tag="ctxrowsb", bufs=2,
                        name=f"crow{b}{half}",
                    )
                    if on_dve:
                        nc.vector.tensor_copy(ctx_row, tailps[0:1, :])
                    else:
                        nc.scalar.copy(out=ctx_row, in_=tailps[0:1, :])
                    # overwrite the same bank: ctx_bc_ps = ones(x)ctx + ones(x)bv
                    nc.tensor.matmul(
                        tailps, onesr_sb, ctx_row, start=True, stop=False
                    )
                    nc.tensor.matmul(
                        tailps, onesr_sb, bvf_sb[0:1, cs], start=False, stop=True
                    )
                    if on_dve:
                        nc.vector.tensor_copy(ctx_bc[:, cs], tailps)
                    else:
                        nc.scalar.copy(out=ctx_bc[:, cs], in_=tailps)
                ctx_bcs[b] = ctx_bc

            def emit_adds_stores(b, split_c=False):
                # residual adds all on DVE (out-of-place, bf16 2x mode);
                # stores as half-supertile DMAs alternating SP/ACT rings.
                # split_c: do each add in two C-halves so the first one can
                # start as soon as ctx_bc's first half is broadcast (used on
                # the last batch where the tail latency is bare).
                fb = fbs[b]
                oview = out_t[b].rearrange("(st p t) c -> st p t c", p=P, t=ST)
                ctx_bc = ctx_bcs[b]
                for st in range(SUP):
                    fsl = fb[:, st * ST : (st + 1) * ST, :]
                    osl = opool.tile([P, ST, C], bf16, tag="ostage")
                    if split_c:
                        for ch in range(2):
                            cs = slice(ch * 512, (ch + 1) * 512)
                            ctx_h = ctx_bc[:, cs].rearrange(
                                "p (o c) -> p o c", o=1
                            ).broadcast_to([P, ST, 512])
                            nc.vector.tensor_add(
                                osl[:, :, cs], fsl[:, :, cs], ctx_h
                            )
                    else:
                        ctx_bc3 = ctx_bc.rearrange(
                            "p (o c) -> p o c", o=1
                        ).broadcast_to([P, ST, C])
                        nc.vector.tensor_add(osl, fsl, ctx_bc3)
                    if split_c and st == SUP - 1:
                        # quarter-tile stores on the very last unit so the
                        # final DMA's transfer+receipt tail is short
                        for q in range(ST):
                            eng = nc.scalar if q % 2 == 0 else nc.sync
                            eng.dma_start(
                                out=oview[st][:, q : q + 1, :],
                                in_=osl[:, q : q + 1, :],
                            )
                    else:
                        for half in range(2):
                            tsl = slice(half * (ST // 2), (half + 1) * (ST // 2))
                            eng = nc.scalar if half == 0 else nc.sync
                            eng.dma_start(
                                out=oview[st][:, tsl, :], in_=osl[:, tsl, :]
                            )

            # pipelined emission: batch-1 loads/stream overlap batch-0 tail;
            # batch-0 adds are slotted into the middle of batch-1's stream
            emit_loads(0)
            emit_stream(0, range(SUP))
            emit_tail(0, use_dve=False)
            emit_loads(1)
            emit_stream(1, range(2))
            emit_adds_stores(0)
            emit_stream(1, range(2, SUP), split_last=True)
            emit_tail(1, use_dve=True)
            emit_adds_stores(1, split_c=True)

    nc.finalize()
    return nc


def _get_program():
    if "nc" not in _CACHE:
        _CACHE["nc"] = _build_program()
    return _CACHE["nc"]


def _prep_in_maps(features, preference, Wq, bq, Wk, Wv, bv):
    import ml_dtypes

    f32 = np.float32
    bf16 = ml_dtypes.bfloat16
    # qk[b,h,:] = (pref[b,h]*Wq[:,0] + bq) @ Wk   -> flat [B, C]
    q = (
        preference.astype(np.float64)[:, :, None] * Wq[:, 0].astype(np.float64)
        + bq.astype(np.float64)
    )  # [B,H,HD]
    qk = np.einsum("bhe,ed->bhd", q, Wk.astype(np.float64))  # [B,H,HD]
    qkflat = np.ascontiguousarray(qk.reshape(B, C)).astype(bf16)
    wvt = np.ascontiguousarray(Wv.T).astype(bf16)
    bvflat = np.ascontiguousarray(np.tile(bv, H)[None, :]).astype(bf16)
    id8 = np.eye(8, dtype=f32).astype(bf16)
    ones128 = np.ones([P, 1], dtype=f32).astype(bf16)
    onesrow = np.ones([1, P], dtype=f32).astype(bf16)
    fbf = np.ascontiguousarray(features).astype(bf16)

    in_maps = []
    for i in range(N_CORES):
        sl = slice(i * BPC, (i + 1) * BPC)
        in_maps.append(
            {
                "features": fbf[sl],
                "qkflat": qkflat[sl],
                "wvt": wvt,
                "bvflat": bvflat,
                "ident8": id8,
                "ones128": ones128,
                "onesrow": onesrow,
            }
        )
    return in_maps


def kernel(features, preference, Wq, bq, Wk, bk, Wv, bv, **_ignored):
    features = np.asarray(features, dtype=np.float32)
    preference = np.asarray(preference, dtype=np.float32)
    Wq = np.asarray(Wq, dtype=np.float32)
    bq = np.asarray(bq, dtype=np.float32)
    Wk = np.asarray(Wk, dtype=np.float32)
    Wv = np.asarray(Wv, dtype=np.float32)
    bv = np.asarray(bv, dtype=np.float32)

    from concourse.bass_utils import run_bass_kernel_spmd

    nc = _get_program()
    in_maps = _prep_in_maps(features, preference, Wq, bq, Wk, Wv, bv)
    res = run_bass_kernel_spmd(nc, in_maps, core_ids=list(range(N_CORES)))
    out = np.concatenate(
        [np.asarray(r["out"]).astype(np.float32) for r in res.results], axis=0
    )
    return out


# revision 18
# speedup vs baseline: 1.0444x; 1.0444x over previous
"""EnhancedDynamicChannelAttention Trainium2 kernel (bf16 datapath, v4).

Reference computation (B=16, S=2048, C=1024, H=8, HD=128):
    q[b,h,:]   = pref[b,h]*Wq[:,0] + bq
    k          = f @ Wk.T + bk ;  v = f @ Wv.T + bv       (per head slice)
    scores     = softmax_s(q . k)                          [B,H,S]
    ctx[b,h,:] = sum_s scores * v[b,s,h,:]                 [B,H,HD]
    out        = f + broadcast_s(ctx)

Algebraic folding (exact up to fp reassociation):
  - softmax shift invariance  -> the q.bk term drops entirely.
  - scores[b,h,s] = f[b,s,h,:] . qk[b,h,:]  with  qk = (pref*Wq+bq) @ Wk
  - sum_s attn = 1  ->  ctx = Wv @ (sum_s attn*f[b,s,h,:]) + bv
  So k/v are never materialized; the kernel is memory bound.

Trace-driven design (v2/v3 perfetto analysis):
  - bf16 datapath end to end (harness gate 2e-2; measured ~5e-3).
  - One HWDGE ring sustains only ~170-210 GB/s, so every f load/store
    is split into half-supertile (0.5 MiB) DMAs alternating between
    the SP and ACT rings; small constants ride the GPSIMD SWDGE ring.
  - DVE and GPSIMD arbitrate one shared SBUF port pair (exclusive
    lock), so ALL elementwise work stays on DVE: tensor_tensor is
    2 elem/cycle in bf16, tensor_reduce always 1 elem/cycle; the
    128-wide segmented reduce is two bf16 TT halvings + 32-wide reduce.
  - 1/sumE folds into the PSUM->SBUF copies (ACT scale operand); bv
    and the ctx partition broadcast are K=1 PE matmuls (no DRAM
    bounce).  Batch-0's tail is ACT-only (DVE is busy streaming
    batch 1); batch-1's tail splits across ACT and DVE chains.
  - Residual adds are DVE out-of-place into a 4-deep staging pool so
    adds never stall on store completion.

Distribution: pure data parallel over batch, 2 batches per core, 8 cores.
"""

import numpy as np

B, S, C = 16, 2048, 1024
H, HD = 8, 128
N_CORES = 8
BPC = B // N_CORES          # batches per core
ST = 4                      # s-rows per partition in a super tile
P = 128
SUP = S // (P * ST)         # super tiles per batch (4)
NT = S // P                 # sub tiles per batch (16)

_CACHE = {}


def _build_program():
    import concourse.bass as bass
    import concourse.bacc as bacc
    import concourse.tile as tile
    from concourse import mybir

    f32 = mybir.dt.float32
    bf16 = mybir.dt.bfloat16

    nc = bacc.Bacc("TRN2", debug=False, num_devices=N_CORES)
    f_in = nc.dram_tensor("features", [BPC, S, C], bf16, kind="ExternalInput")
    qk_in = nc.dram_tensor("qkflat", [BPC, C], bf16, kind="ExternalInput")
    wvt_in = nc.dram_tensor("wvt", [HD, HD], bf16, kind="ExternalInput")
    bvf_in = nc.dram_tensor("bvflat", [1, C], bf16, kind="ExternalInput")
    id8_in = nc.dram_tensor("ident8", [8, 8], bf16, kind="ExternalInput")
    ones_in = nc.dram_tensor("ones128", [P, 1], bf16, kind="ExternalInput")
    onesr_in = nc.dram_tensor("onesrow", [1, P], bf16, kind="ExternalInput")
    out_t = nc.dram_tensor("out", [BPC, S, C], bf16, kind="ExternalOutput")

    with tile.TileContext(nc) as tc, nc.allow_low_precision(
        reason="bf16 datapath; rel-err budget 2e-2"
    ):
        with (
            tc.tile_pool(name="fpool", bufs=BPC) as fpool,
            tc.tile_pool(name="tmppool", bufs=2) as tmppool,
            tc.tile_pool(name="opool", bufs=4) as opool,
            tc.tile_pool(name="spool", bufs=2 * SUP) as spool,
            tc.tile_pool(name="small", bufs=2) as small,
            tc.tile_pool(name="singles", bufs=1) as singles,
            tc.tile_pool(name="ps1", bufs=1, space="PSUM") as ps1,
            tc.tile_pool(name="ps2", bufs=2, space="PSUM") as ps2,
        ):
            # tiny loads first: onesrow + qk rows (single-descriptor DMAs);
            # the DMA-broadcast of qk (128 dup reads) is slow on any ring,
            # so broadcast on-chip with K=1 PE matmuls instead.  Other
            # constants ride the GPSIMD SWDGE ring.
            onesr_sb = singles.tile([1, P], bf16)
            nc.gpsimd.dma_start(out=onesr_sb, in_=onesr_in[:, :])
            qk_rows = []
            for b in range(BPC):
                qk_row = small.tile([1, C], bf16, tag="qkrow", name=f"qkr{b}")
                nc.gpsimd.dma_start(out=qk_row, in_=qk_in[b : b + 1, :])
                qk_rows.append(qk_row)

            ones_sb = singles.tile([P, 1], bf16)
            nc.gpsimd.dma_start(out=ones_sb, in_=ones_in[:, :])
            wvt_sb = singles.tile([HD, HD], bf16)
            nc.gpsimd.dma_start(out=wvt_sb, in_=wvt_in[:, :])
            id8_sb = singles.tile([8, 8], bf16)
            nc.gpsimd.dma_start(out=id8_sb, in_=id8_in[:, :])
            bvf_sb = singles.tile([1, C], bf16)
            nc.gpsimd.dma_start(out=bvf_sb, in_=bvf_in[:, :])

            # qk_bc[p, c] = qk[c] for all p, via ones (x) row PE matmuls
            qk_bcs = []
            for b in range(BPC):
                qk_bc = small.tile([P, C], bf16, tag="qkbc", name=f"qkbc{b}")
                for half in range(2):
                    cs = slice(half * 512, (half + 1) * 512)
                    qps = ps1.tile(
                        [P, 512], f32, tag="tailps", bufs=2, name=f"qps{b}{half}"
                    )
                    nc.tensor.matmul(
                        qps, onesr_sb, qk_rows[b][0:1, cs], start=True, stop=True
                    )
                    nc.scalar.copy(out=qk_bc[:, cs], in_=qps)
                qk_bcs.append(qk_bc)

            fbs = [
                fpool.tile([P, NT, C], bf16, tag="fb", name=f"fb{b}")
                for b in range(BPC)
            ]
            uwfAs, uwfBs, sumEs = [], [], []
            for b in range(BPC):
                uwfAs.append(
                    ps2.tile([P, 512], f32, tag="uwfA", bufs=2, name=f"uwfA{b}")
                )
                uwfBs.append(
                    ps2.tile([P, 512], f32, tag="uwfB", bufs=2, name=f"uwfB{b}")
                )
                sumEs.append(
                    ps2.tile([8, 1], f32, tag="sumE", bufs=1, name=f"sumE{b}")
                )
            ctx_bcs = [None] * BPC

            def emit_loads(b):
                # half-supertile (0.5 MiB) DMAs alternating SP/ACT rings:
                # one HWDGE queue alone tops out ~200 GB/s
                fview = f_in[b].rearrange("(st p t) c -> st p t c", p=P, t=ST)
                for st in range(SUP):
                    for half in range(2):
                        lo = st * ST + half * (ST // 2)
                        eng = nc.sync if half == 0 else nc.scalar
                        eng.dma_start(
                            out=fbs[b][:, lo : lo + ST // 2, :],
                            in_=fview[st][
                                :, half * (ST // 2) : (half + 1) * (ST // 2), :
                            ],
                        )

            def emit_stream_unit(b, t_lo, t_hi, first, last):
                # one pipeline unit covering fb rows [t_lo, t_hi)
                nt = t_hi - t_lo
                qk_bc3 = qk_bcs[b].rearrange(
                    "p (o c) -> p o c", o=1
                ).broadcast_to([P, nt, C])
                fb = fbs[b]
                uwfA, uwfB, sumE = uwfAs[b], uwfBs[b], sumEs[b]
                tmp = tmppool.tile([P, nt, C], bf16, tag="tmp")
                nc.vector.tensor_mul(tmp, fb[:, t_lo:t_hi, :], qk_bc3)
                tmp4 = tmp.rearrange("p t (h d) -> p t h d", h=H)
                # segmented reduce over d=128: two packed bf16 halvings
                # (2 elem/cycle) + one 32-wide 1x reduce
                h1 = tmppool.tile([P, nt, H, HD // 2], bf16, tag="h1")
                nc.vector.tensor_add(
                    h1, tmp4[:, :, :, 0 : HD // 2], tmp4[:, :, :, HD // 2 : HD]
                )
                h2 = tmppool.tile([P, nt, H, HD // 4], bf16, tag="h2")
                nc.vector.tensor_add(
                    h2, h1[:, :, :, 0 : HD // 4], h1[:, :, :, HD // 4 : HD // 2]
                )
                h3 = tmppool.tile([P, nt, H, HD // 8], bf16, tag="h3")
                nc.vector.tensor_add(
                    h3, h2[:, :, :, 0 : HD // 8], h2[:, :, :, HD // 8 : HD // 4]
                )
                scores = spool.tile([P, nt, H], f32, tag="scores")
                nc.vector.reduce_sum(scores, h3, axis=mybir.AxisListType.X)
                E_sup = spool.tile([P, nt, H], bf16, tag="esup")
                nc.scalar.activation(
                    out=E_sup.rearrange("p t h -> p (t h)"),
                    in_=scores.rearrange("p t h -> p (t h)"),
                    func=mybir.ActivationFunctionType.Exp,
                )
                for t in range(nt):
                    mm_first = first and t == 0
                    mm_last = last and t == nt - 1
                    e_sl = E_sup[:, t, :]
                    f_sl = fb[:, t_lo + t, :]
                    nc.tensor.matmul(
                        uwfA[0:8, :], e_sl, f_sl[:, 0:512],
                        start=mm_first, stop=mm_last,
                    )
                    nc.tensor.matmul(
                        uwfB[0:8, :], e_sl, f_sl[:, 512:1024],
                        start=mm_first, stop=mm_last,
                    )
                    nc.tensor.matmul(
                        sumE, e_sl, ones_sb, start=mm_first, stop=mm_last
                    )

            def emit_stream(b, st_range, split_last=False):
                for st in st_range:
                    last_st = st == SUP - 1
                    if split_last and last_st:
                        # halve the final unit so the tail's PE drain starts
                        # ~2us earlier
                        emit_stream_unit(
                            b, st * ST, st * ST + ST // 2, st == 0, False
                        )
                        emit_stream_unit(
                            b, st * ST + ST // 2, (st + 1) * ST, False, True
                        )
                    else:
                        emit_stream_unit(
                            b, st * ST, (st + 1) * ST, st == 0, last_st
                        )

            def emit_tail(b, use_dve):
                # wf = diag-blocks(uwf)/sumE; ctx_row = wf @ WvT + bv;
                # partition-broadcast via K=1 PE matmuls (ones (x) row).
                # use_dve=True splits the two C-halves across ACT and DVE
                # chains (only when DVE is otherwise idle, i.e. last batch).
                uwfA, uwfB, sumE = uwfAs[b], uwfBs[b], sumEs[b]
                recip = small.tile([8, 1], f32, tag="recip", name=f"recip{b}")
                nc.vector.reciprocal(recip, sumE)
                uwf_sb = small.tile([8, C], bf16, tag="uwfsb", bufs=1)
                nc.scalar.mul(uwf_sb[:, 0:512], uwfA[0:8, :], recip)
                if use_dve:
                    nc.vector.tensor_scalar_mul(
                        uwf_sb[:, 512:1024], uwfB[0:8, :], recip
                    )
                else:
                    nc.scalar.mul(uwf_sb[:, 512:1024], uwfB[0:8, :], recip)
                # per-head PE transpose into [128, 8*8]; diagonal columns
                # (stride 9) hold wfT[d, h] = uwf[h, h*128+d] / sumE[h]
                wfT8_ps = ps1.tile([P, H * H], bf16, tag="wft8")
                for h in range(H):
                    nc.tensor.transpose(
                        wfT8_ps[:, h * H : (h + 1) * H],
                        uwf_sb[:, h * HD : (h + 1) * HD],
                        id8_sb,
                    )
                wfT8_sb = small.tile([P, H * H], bf16, tag="wft8sb", bufs=1)
                nc.scalar.copy(out=wfT8_sb, in_=wfT8_ps)

                ctx_bc = small.tile([P, C], bf16, tag="ctxbc", name=f"ctxbc{b}")
                for half in range(2):
                    cs = slice(half * 512, (half + 1) * 512)
                    on_dve = use_dve and half == 1
                    tailps = ps1.tile(
                        [P, 512], f32, tag="tailps", bufs=2, name=f"tps{b}{half}"
                    )
                    for hh in range(4):
                        h = half * 4 + hh
                        nc.tensor.matmul(
                            tailps[0:1, hh * HD : (hh + 1) * HD],
                            wfT8_sb[:, h * (H + 1) : h * (H + 1) + 1],
                            wvt_sb,
                            start=True,
                            stop=True,
                        )
                    ctx_row = small.tile(
                        [1, 512], bf16, tag="ctxrowsb", bufs=2,
                        name=f"crow{b}{half}",
                    )
                    if on_dve:
                        nc.vector.tensor_copy(ctx_row, tailps[0:1, :])
                    else:
                        nc.scalar.copy(out=ctx_row, in_=tailps[0:1, :])
                    # overwrite the same bank: ctx_bc_ps = ones(x)ctx + ones(x)bv
                    nc.tensor.matmul(
                        tailps, onesr_sb, ctx_row, start=True, stop=False
                    )
                    nc.tensor.matmul(
                        tailps, onesr_sb, bvf_sb[0:1, cs], start=False, stop=True
                    )
                    if on_dve:
                        nc.vector.tensor_copy(ctx_bc[:, cs], tailps)
                    else:
                        nc.scalar.copy(out=ctx_bc[:, cs], in_=tailps)
                ctx_bcs[b] = ctx_bc

            def emit_adds_stores(b, split_c=False):
                # residual adds all on DVE (out-of-place, bf16 2x mode);
                # stores as half-supertile DMAs alternating SP/ACT rings.
                # split_c: do each add in two C-halves so the first one can
                # start as soon as ctx_bc's first half is broadcast (used on
                # the last batch where the tail latency is bare).
                fb = fbs[b]
                oview = out_t[b].rearrange("(st p t) c -> st p t c", p=P, t=ST)
                ctx_bc = ctx_bcs[b]
                for st in range(SUP):
                    fsl = fb[:, st * ST : (st + 1) * ST, :]
                    osl = opool.tile([P, ST, C], bf16, tag="ostage")
                    if split_c:
                        for ch in range(2):
                            cs = slice(ch * 512, (ch + 1) * 512)
                            ctx_h = ctx_bc[:, cs].rearrange(
                                "p (o c) -> p o c", o=1
                            ).broadcast_to([P, ST, 512])
                            nc.vector.tensor_add(
                                osl[:, :, cs], fsl[:, :, cs], ctx_h
                            )
                    else:
                        ctx_bc3 = ctx_bc.rearrange(
                            "p (o c) -> p o c", o=1
                        ).broadcast_to([P, ST, C])
                        nc.vector.tensor_add(osl, fsl, ctx_bc3)
                    for half in range(2):
                        tsl = slice(half * (ST // 2), (half + 1) * (ST // 2))
                        eng = nc.scalar if half == 0 else nc.sync
                        eng.dma_start(
                            out=oview[st][:, tsl, :], in_=osl[:, tsl, :]
                        )

            # pipelined emission: batch-1 loads/stream overlap batch-0 tail;
            # batch-0 adds are slotted into the middle of batch-1's stream
            emit_loads(0)
            emit_stream(0, range(SUP))
            emit_tail(0, use_dve=False)
            emit_loads(1)
            emit_stream(1, range(2))
            emit_adds_stores(0)
            emit_stream(1, range(2, SUP), split_last=True)
            emit_tail(1, use_dve=True)
            emit_adds_stores(1, split_c=True)

    nc.finalize()
    return nc


def _get_program():
    if "nc" not in _CACHE:
        _CACHE["nc"] = _build_program()
    return _CACHE["nc"]


def _prep_in_maps(features, preference, Wq, bq, Wk, Wv, bv):
    import ml_dtypes

    f32 = np.float32
    bf16 = ml_dtypes.bfloat16
    # qk[b,h,:] = (pref[b,h]*Wq[:,0] + bq) @ Wk   -> flat [B, C]
    q = (
        preference.astype(np.float64)[:, :, None] * Wq[:, 0].astype(np.float64)
        + bq.astype(np.float64)
    )  # [B,H,HD]
    qk = np.einsum("bhe,ed->bhd", q, Wk.astype(np.float64))  # [B,H,HD]
    qkflat = np.ascontiguousarray(qk.reshape(B, C)).astype(bf16)
    wvt = np.ascontiguousarray(Wv.T).astype(bf16)
    bvflat = np.ascontiguousarray(np.tile(bv, H)[None, :]).astype(bf16)
    id8 = np.eye(8, dtype=f32).astype(bf16)
    ones128 = np.ones([P, 1], dtype=f32).astype(bf16)
    onesrow = np.ones([1, P], dtype=f32).astype(bf16)
    fbf = np.ascontiguousarray(features).astype(bf16)

    in_maps = []
    for i in range(N_CORES):
        sl = slice(i * BPC, (i + 1) * BPC)
        in_maps.append(
            {
                "features": fbf[sl],
                "qkflat": qkflat[sl],
                "wvt": wvt,
                "bvflat": bvflat,
                "ident8": id8,
                "ones128": ones128,
                "onesrow": onesrow,
            }
        )
    return in_maps


def kernel(features, preference, Wq, bq, Wk, bk, Wv, bv, **_ignored):
    features = np.asarray(features, dtype=np.float32)
    preference = np.asarray(preference, dtype=np.float32)
    Wq = np.asarray(Wq, dtype=np.float32)
    bq = np.asarray(bq, dtype=np.float32)
    Wk = np.asarray(Wk, dtype=np.float32)
    Wv = np.asarray(Wv, dtype=np.float32)
    bv = np.asarray(bv, dtype=np.float32)

    from concourse.bass_utils import run_bass_kernel_spmd

    nc = _get_program()
    in_maps = _prep_in_maps(features, preference, Wq, bq, Wk, Wv, bv)
    res = run_bass_kernel_spmd(nc, in_maps, core_ids=list(range(N_CORES)))
    out = np.concatenate(
        [np.asarray(r["out"]).astype(np.float32) for r in res.results], axis=0
    )
    return out
